# revision 2
# baseline (speedup 1.0000x reference)
"""Additive (Bahdanau) attention on Trainium2, data-parallel over batch on 8 NeuronCores.

Math (per batch b):
    qp = queries @ W_q                     [Tq, H]
    kp = keys @ W_k + b                    [Tk, H]
    scores[q,k] = sum_h v[h] * tanh(qp[q,h] + kp[k,h])
    masked softmax over k (k < seq_len[b]), then out = align @ keys.

Design (per core, 4 batch "slots" with compile-time key-lengths L_slots):
  - keys/queries transposed on PE (identity matmul), projections on PE (bias b
    folded into the projection via an appended ones-row / W_k||b const).
  - kpb duplicated across both 64-partition halves -> kpb2 [128=2h, L].
  - S[h2, j*L+k] = kpb2 + qp2[:, j] per query-pair j via DVE tensor_scalar adds
    (bf16, 4x mode), tanh on ACT in two big ops.
  - scores^T[k, q] via PE matmuls: lhsT = tanh tile [128h2, <=128k] (stationary),
    rhs = v2blk [128, 2] -> psum [k, 2q] per query pair; block-diagonal v gives
    both queries of a pair in one matmul.
  - exp on ACT directly from PSUM with per-partition bias = 0 / -30000 mask
    column (folds sequence masking into the exp).
  - final: out_un[q, h] | rowsum = E^T-chunks (lhsT) @ [keys || ones] (rhs),
    PSUM-accumulated over k-chunks; divide by rowsum via DVE reciprocal+scale.

Batches are sorted by seq_len and dealt so each core gets one batch per slot
rank; slot k-length = max over the 8 batches of that rank (padded to 8). All
cores run the identical program on different data (SPMD).
"""

import sys

_REPO = "/opt/trn_rl_repo"
if _REPO not in sys.path:
    sys.path.insert(0, _REPO)

import numpy as np
import ml_dtypes

from concourse import bacc, tile
import concourse.mybir as mybir
from concourse import bass_utils

B, TQ, TK, H = 32, 64, 256, 64
NCORES = 8
SLOTS = 4
F32 = mybir.dt.float32
BF16 = mybir.dt.bfloat16
TANH = mybir.ActivationFunctionType.Tanh
EXP = mybir.ActivationFunctionType.Exp
MASK_NEG = -30000.0

_prog_cache: dict = {}


def _roundup(x, m):
    return ((x + m - 1) // m) * m


def _chunks(L):
    out, off = [], 0
    while off < L:
        w = min(128, L - off)
        out.append((off, w))
        off += w
    return out


def _build(L_slots):
    nc = bacc.Bacc(
        "TRN2",
        target_bir_lowering=False,
        debug=False,
        enable_asserts=False,
        num_devices=NCORES,
    )
    q_d = nc.dram_tensor("queries", [SLOTS, TQ, H], F32, kind="ExternalInput").ap()
    k_d = nc.dram_tensor("keys", [SLOTS, TK, H], F32, kind="ExternalInput").ap()
    mb_d = nc.dram_tensor("maskbias", [SLOTS, TK], F32, kind="ExternalInput").ap()
    wq_d = nc.dram_tensor("wq", [H, H], F32, kind="ExternalInput").ap()
    wkb_d = nc.dram_tensor("wkb", [H + 1, H], F32, kind="ExternalInput").ap()
    v2_d = nc.dram_tensor("v2", [128, 2], BF16, kind="ExternalInput").ap()
    id_d = nc.dram_tensor("ident", [128, 128], F32, kind="ExternalInput").ap()
    o_d = nc.dram_tensor("out", [SLOTS, TQ, H], F32, kind="ExternalOutput").ap()

    with tile.TileContext(nc) as tc:
        with (
            tc.tile_pool(name="const", bufs=1) as cpool,
            tc.tile_pool(name="qpool", bufs=1) as qpool,
            tc.tile_pool(name="kpool", bufs=2) as kpool,
            tc.tile_pool(name="wpool", bufs=3) as wpool,
            tc.tile_pool(name="spool", bufs=2) as spool,
            tc.tile_pool(name="tpp", bufs=2, space="PSUM") as tpp,
            tc.tile_pool(name="prj", bufs=2, space="PSUM") as prj,
            tc.tile_pool(name="scp", bufs=3, space="PSUM") as scp,
            tc.tile_pool(name="oup", bufs=1, space="PSUM") as oup,
        ):
            wq_sb = cpool.tile([H, H], F32, name="wq_sb", tag="wq")
            nc.sync.dma_start(out=wq_sb, in_=wq_d)
            wkb_sb = cpool.tile([H + 1, H], F32, name="wkb_sb", tag="wkb")
            nc.sync.dma_start(out=wkb_sb, in_=wkb_d)
            v2_sb = cpool.tile([128, 2], BF16, name="v2_sb", tag="v2")
            nc.sync.dma_start(out=v2_sb, in_=v2_d)
            id_sb = cpool.tile([128, 128], F32, name="id_sb", tag="id")
            nc.sync.dma_start(out=id_sb, in_=id_d)

            # Tiny activation up front so the ACT table set loads while the
            # first projections run.
            scr = cpool.tile([1, 2], F32, name="scr", tag="scr")
            nc.vector.memset(scr, 0.0)
            nc.scalar.activation(scr, scr, TANH)

            # queries: transpose + project, two slots at a time
            qp2g = []
            for g in range(2):
                qnat = wpool.tile([128, H], F32, name=f"qnat{g}", tag="qnat")
                nc.sync.dma_start(out=qnat, in_=q_d[2 * g : 2 * g + 2])
                qT_ps = tpp.tile([H, 128], F32, name=f"qTps{g}", tag="tp")
                nc.tensor.transpose(qT_ps, qnat, id_sb)
                qT_sb = wpool.tile([H, 128], F32, name=f"qTsb{g}", tag="qT")
                nc.vector.tensor_copy(qT_sb, qT_ps)
                qpT_ps = prj.tile([H, 128], F32, name=f"qpTps{g}", tag="prj")
                nc.tensor.matmul(qpT_ps, lhsT=wq_sb, rhs=qT_sb)
                # qp2[0:64, j] = qpT[:, 2j], qp2[64:128, j] = qpT[:, 2j+1]
                qp2 = qpool.tile([128, 64], F32, name=f"qp2_{g}", tag=f"qp2_{g}")
                nc.vector.tensor_copy(qp2[0:64, :], qpT_ps[:, 0:128:2])
                nc.vector.tensor_copy(qp2[64:128, :], qpT_ps[:, 1:128:2])
                qp2g.append(qp2)

            for s in range(SLOTS):
                L = L_slots[s]
                chs = _chunks(L)
                nch = len(chs)

                knat = []
                for ci, (off, w) in enumerate(chs):
                    t = kpool.tile([128, H + 1], F32, name=f"knat{s}_{ci}", tag=f"knat{ci}")
                    nc.sync.dma_start(out=t[0:w, 0:H], in_=k_d[s, off : off + w, :])
                    nc.gpsimd.memset(t[0:w, H : H + 1], 1.0)
                    knat.append(t)

                mbs = []
                for ci, (off, w) in enumerate(chs):
                    t = wpool.tile([128, 1], F32, name=f"mb{s}_{ci}", tag=f"mb{ci}")
                    nc.sync.dma_start(out=t[0:w, :], in_=mb_d[s, off : off + w])
                    mbs.append(t)

                keysT = kpool.tile([H + 1, TK], F32, name=f"keysT{s}", tag="keysT")
                for ci, (off, w) in enumerate(chs):
                    kT_ps = tpp.tile([H, 128], F32, name=f"kTps{s}_{ci}", tag="tp")
                    nc.tensor.transpose(
                        kT_ps[0:H, 0:w], knat[ci][0:w, 0:H], id_sb[0:w, 0:w]
                    )
                    nc.vector.tensor_copy(keysT[0:H, off : off + w], kT_ps[0:H, 0:w])
                nc.gpsimd.memset(keysT[H : H + 1, 0:L], 1.0)

                kpT_ps = prj.tile([H, TK], F32, name=f"kpTps{s}", tag="prj")
                nc.tensor.matmul(
                    kpT_ps[0:H, 0:L], lhsT=wkb_sb, rhs=keysT[:, 0:L]
                )
                kpb2 = wpool.tile([128, TK], BF16, name=f"kpb2_{s}", tag="kpb2")
                nc.vector.tensor_copy(kpb2[0:64, 0:L], kpT_ps[0:H, 0:L])
                nc.vector.tensor_copy(kpb2[64:128, 0:L], kpb2[0:64, 0:L])

                qp2 = qp2g[s // 2]
                qoff = 32 * (s % 2)
                S_all = spool.tile([128, 32 * L], BF16, name=f"S{s}", tag="S")
                for j in range(32):
                    nc.vector.tensor_scalar_add(
                        S_all[:, j * L : (j + 1) * L],
                        kpb2[:, 0:L],
                        qp2[:, qoff + j : qoff + j + 1],
                    )
                S_tanh = spool.tile([128, 32 * L], BF16, name=f"T{s}", tag="T")
                half = 16 * L
                nc.scalar.activation(S_tanh[:, 0:half], S_all[:, 0:half], TANH)
                nc.scalar.activation(S_tanh[:, half : 32 * L], S_all[:, half : 32 * L], TANH)

                out_ps = oup.tile([TQ, H + 1], F32, name=f"ops{s}", tag="ou")
                for ci, (off, w) in enumerate(chs):
                    sc_ps = scp.tile([128, TQ], F32, name=f"sc{s}_{ci}", tag="sc")
                    for j in range(32):
                        nc.tensor.matmul(
                            sc_ps[0:w, 2 * j : 2 * j + 2],
                            lhsT=S_tanh[:, j * L + off : j * L + off + w],
                            rhs=v2_sb,
                            start=True,
                            stop=True,
                        )
                    E = wpool.tile([128, TQ], F32, name=f"E{s}_{ci}", tag=f"E{ci}")
                    nc.scalar.activation(
                        E[0:w, :], sc_ps[0:w, :], EXP, bias=mbs[ci][0:w, :]
                    )
                    nc.tensor.matmul(
                        out_ps,
                        lhsT=E[0:w, 0:TQ],
                        rhs=knat[ci][0:w, 0 : H + 1],
                        start=(ci == 0),
                        stop=(ci == nch - 1),
                    )

                recip = wpool.tile([TQ, 1], F32, name=f"rc{s}", tag="rc")
                nc.vector.reciprocal(recip, out_ps[:, H : H + 1])
                out_sb = wpool.tile([TQ, H], F32, name=f"osb{s}", tag="osb")
                nc.vector.tensor_scalar_mul(out_sb, out_ps[:, 0:H], recip)
                nc.sync.dma_start(out=o_d[s], in_=out_sb)

    nc.compile()
    return nc


def _get_prog(L_slots):
    if L_slots not in _prog_cache:
        _prog_cache[L_slots] = _build(L_slots)
    return _prog_cache[L_slots]


def _plan(seq_len_flat):
    sl = np.asarray(seq_len_flat).reshape(-1).astype(np.int64)
    order = np.argsort(-sl, kind="stable")
    assign = np.zeros((NCORES, SLOTS), dtype=np.int64)
    L_slots = []
    for s in range(SLOTS):
        grp = order[NCORES * s : NCORES * (s + 1)]
        assign[:, s] = grp
        L = int(max(1, sl[grp].max()))
        L_slots.append(min(TK, _roundup(L, 8)))
    return tuple(L_slots), assign, sl


def _make_in_maps(queries, keys, sl, assign, W_q, W_k, v, b):
    wkb = np.concatenate(
        [W_k.astype(np.float32), b.reshape(1, H).astype(np.float32)], axis=0
    )
    v2 = np.zeros((128, 2), dtype=ml_dtypes.bfloat16)
    vv = np.asarray(v, dtype=np.float32).reshape(-1)
    v2[0:64, 0] = vv.astype(ml_dtypes.bfloat16)
    v2[64:128, 1] = vv.astype(ml_dtypes.bfloat16)
    ident = np.eye(128, dtype=np.float32)
    wq = np.asarray(W_q, dtype=np.float32)

    in_maps = []
    for c in range(NCORES):
        bidx = assign[c]
        mbias = np.zeros((SLOTS, TK), np.float32)
        for s_i, b_i in enumerate(bidx):
            mbias[s_i, sl[b_i] :] = MASK_NEG
        in_maps.append(
            {
                "queries": np.ascontiguousarray(queries[bidx]),
                "keys": np.ascontiguousarray(keys[bidx]),
                "maskbias": mbias,
                "wq": wq,
                "wkb": wkb,
                "v2": v2,
                "ident": ident,
            }
        )
    return in_maps


def _run_spmd(nc, in_maps, trace=False, trace_kwargs=None):
    from concourse.bass_interp import get_hw_module

    old = nc.m
    nc.m = get_hw_module(nc.m)
    try:
        res = bass_utils.run_bass_kernel_spmd(
            nc,
            in_maps,
            core_ids=list(range(NCORES)),
            trace=trace,
            **(trace_kwargs or {}),
        )
    finally:
        nc.m = old
    return res


def kernel(queries, keys, seq_len, W_q, W_k, v, b, _trace=False):
    queries = np.asarray(queries, dtype=np.float32)
    keys = np.asarray(keys, dtype=np.float32)
    L_slots, assign, sl = _plan(seq_len)
    nc = _get_prog(L_slots)
    in_maps = _make_in_maps(queries, keys, sl, assign, W_q, W_k, v, b)
    res = _run_spmd(nc, in_maps, trace=_trace)
    out = np.zeros((B, TQ, H), np.float32)
    for c in range(NCORES):
        o = res.results[c]["out"]
        for s_i, b_i in enumerate(assign[c]):
            out[b_i] = o[s_i]
    if _trace:
        kernel._last_results = res
    return out
